# revision 9
# baseline (speedup 1.0000x reference)
"""Fused LayerNorm + multi-head attention + output projection on 8 TRN2 cores.

Sharding: core c handles batch b = c//4 and head group g = c%4 (4 of 16 heads).
Each core computes LN(x[b]) (replicated within the batch's 4 cores), the qkv
projection for its heads, attention, and a partial output projection (w_out
rows for its heads). The host sums the 4 partials per batch.

On-chip layout is fully transposed ([feature, token]); the host pre-transposes
x, folds gamma / softmax scale / beta into the weights, and packs everything in
SBUF-ready layouts, so the kernel needs zero on-chip transposes:

  xn^T   [D, T]   = LayerNorm(x)^T        (stats via ones-matmul broadcast,
                                           tb-pipelined with the qkv matmuls)
  q^T/k^T [dh, T] = W_q/k^T-slices @ xn^T (feature-major)
  v      [T, dh]  = xn^T-tiles.T @ W_v    (token-major, swapped operands;
                                           v-bias folded into the V columns)
  E^T    [k, q]   = exp(K Q^T)            (no max subtraction: scores ~N(0,1))
  av^T   [dh, q]  = V-block @ E^T         (den rows ride along: even head ->
                                           psum row 127, odd head -> row 0)
  out^T  [D, T]   = w_out-slice^T @ (av^T/den)   (2 heads packed per 128 rows)
"""

import numpy as np

HEADS = 16
DIM_HEAD = 64
SCALE = DIM_HEAD**-0.5
EPS = 1e-5
B, S, D = 2, 2048, 1024
T = S
NCORES = 8
NH = 4  # heads per core
F = 3 * NH * DIM_HEAD  # 768 features per core: [q(256) | k(256) | v(256)]
DC = D // 128  # 8 contraction chunks
KC = T // 128  # 16 key chunks
QB = 4  # q blocks
QW = T // QB  # 512 q block width
WQK = 4 * DC * 128  # 4096: m-major q/k region of the wqkv tile

_cache = {}


def _build():
    import concourse.bacc as bacc
    import concourse.mybir as mybir
    import concourse.tile as tile

    fp32 = mybir.dt.float32
    bf16 = mybir.dt.bfloat16
    AF = mybir.ActivationFunctionType
    ALU = mybir.AluOpType

    nc = bacc.Bacc("TRN2", target_bir_lowering=False, debug=False,
                   num_devices=NCORES)
    # wqkv layout: [q/k m-major: (m c j) 4*8*128 | v c-major: (c j) 8*256]
    xt_d = nc.declare_dram_parameter("xt", [128, DC * T], bf16, isOutput=False)
    wqkv_d = nc.declare_dram_parameter("wqkv", [128, DC * F], bf16, isOutput=False)
    wout_d = nc.declare_dram_parameter("wout", [128, 2 * D], bf16, isOutput=False)
    bqkc_d = nc.declare_dram_parameter("bqkc", [128, 4], fp32, isOutput=False)
    vbrow_d = nc.declare_dram_parameter("vbrow", [1, 1024], bf16, isOutput=False)
    bout_d = nc.declare_dram_parameter("bout", [128, 8], fp32, isOutput=False)
    out_d = nc.declare_dram_parameter("out", [D, T], bf16, isOutput=True)
    dbg = {}
    if _cache.get("debug"):
        dbg["xn"] = nc.declare_dram_parameter("dbg_xn", [128, DC * T], bf16, isOutput=True)
        dbg["qk"] = nc.declare_dram_parameter("dbg_qk", [128, 4 * T], bf16, isOutput=True)
        dbg["vsb"] = nc.declare_dram_parameter("dbg_vsb", [128, KC * NH * 128], bf16, isOutput=True)
        dbg["aot"] = nc.declare_dram_parameter("dbg_aot", [128, 2 * T], bf16, isOutput=True)

    with tile.TileContext(nc) as tc:
        with (
            tc.tile_pool(name="const", bufs=1) as constp,
            tc.tile_pool(name="big", bufs=1) as bigp,
            tc.tile_pool(name="work", bufs=2) as workp,
            tc.tile_pool(name="psum", bufs=1, space="PSUM") as psump,
        ):
            # ---- persistent SBUF ----
            ones128 = constp.tile([128, 128], bf16, tag="ones128")
            nc.gpsimd.memset(ones128[:], 1.0)
            wqkv = constp.tile([128, DC * F], bf16, tag="wqkv")
            wout = constp.tile([128, 2 * D], bf16, tag="wout")
            bqkc = constp.tile([128, 4], fp32, tag="bqkc")
            bout = constp.tile([128, 8], fp32, tag="bout")
            vbrow = constp.tile([1, 1024], bf16, tag="vbrow")
            vb = constp.tile([128, 1024], bf16, tag="vb")

            xn = bigp.tile([128, DC * T], bf16, tag="xn")  # normalized x^T
            mean_b = bigp.tile([128, T], bf16, tag="mean_b")
            rstd_b = bigp.tile([128, T], bf16, tag="rstd_b")
            # q^T / k^T feature-major: m=0,1 -> q heads (0,1),(2,3); m=2,3 -> k
            qk = bigp.tile([128, 4 * T], bf16, tag="qk")
            # v blocks, 128 wide per (k-chunk, head), all heads alike:
            #   [one@0 | zeros(63) | V(64)@64:128] -> den at av row 0
            vsb = bigp.tile([128, KC * NH * 128], bf16, tag="vsb")
            nc.gpsimd.memset(vsb[:], 0.0)
            vsb_r = vsb[:].rearrange("p (c h o) -> p c h o", h=NH, o=128)
            nc.gpsimd.memset(vsb_r[:, :, :, 0:1], 1.0)
            # attention output^T, packed: chunk hh//2; odd head -> rows 0:64,
            # even head -> rows 64:128 (wout_sb rows swapped to match)
            aot = bigp.tile([128, 2 * T], bf16, tag="aot")

            # psum: A = scores/phase-1 qkv [128,2048] (4 banks),
            #       C = AV / phase-1 stats (2), D = fillers / stats (2)
            ps_n = [0]

            def ps(tag, width):
                ps_n[0] += 1
                return psump.tile([128, width], fp32, tag=tag,
                                  name=f"ps_{tag}_{ps_n[0]}")

            # ---- input DMAs, interleaved for just-in-time arrival ----
            lnp = tc.tile_pool(name="ln", bufs=1)
            lnp_pool = lnp.__enter__()
            xtb = lnp_pool.tile([128, DC * T], bf16, tag="xtb")

            def dma_xt(tb):
                for c in range(DC):
                    sl = slice(c * T + tb * 512, c * T + (tb + 1) * 512)
                    nc.sync.dma_start(xtb[:, sl], xt_d[:, sl])

            def dma_w(lo, hi):
                nc.sync.dma_start(wqkv[:, lo:hi], wqkv_d[:, lo:hi])

            dma_xt(0)
            dma_w(2 * 1024, 3 * 1024)  # m=2 (k heads 0,1)
            dma_w(0 * 1024, 1 * 1024)  # m=0 (q heads 0,1)
            dma_xt(1)
            dma_w(3 * 1024, 4 * 1024)  # m=3
            dma_w(1 * 1024, 2 * 1024)  # m=1
            dma_xt(2)
            nc.sync.dma_start(wqkv[:, WQK:], wqkv_d[:, WQK:])  # v
            dma_xt(3)
            nc.sync.dma_start(wout[:], wout_d[:])
            nc.sync.dma_start(bqkc[:], bqkc_d[:])
            nc.sync.dma_start(bout[:], bout_d[:])
            nc.sync.dma_start(vbrow[:], vbrow_d[:])
            nc.gpsimd.partition_broadcast(vb[:], vbrow[0:1, :])

            # ---- qkv building blocks ----
            def qk_unit(m, tb, slot):
                # q/k projection for one (m-slice, token-block): [128, 512]
                tsl = slice(tb * 512, (tb + 1) * 512)
                for c in range(DC):
                    nc.tensor.matmul(
                        slot,
                        wqkv[:, (m * DC + c) * 128:(m * DC + c + 1) * 128],
                        xn[:, c * T + tb * 512:c * T + (tb + 1) * 512],
                        start=(c == 0), stop=(c == DC - 1))
                nc.vector.tensor_scalar(
                    out=qk[:, m * T + tb * 512:m * T + (tb + 1) * 512],
                    in0=slot, scalar1=bqkc[:, m:m + 1], scalar2=None,
                    op0=ALU.add)

            def v_unit(tq, slot):
                # v for 4 token-tiles (512 tokens), token-major [tok, (h d)]
                for half in range(4):
                    tt = tq * 4 + half
                    o = slot[:, half * 256:(half + 1) * 256]
                    for c in range(DC):
                        nc.tensor.matmul(
                            o,
                            xn[:, c * T + tt * 128:c * T + (tt + 1) * 128],
                            wqkv[:, WQK + c * 256:WQK + (c + 1) * 256],
                            start=(c == 0), stop=(c == DC - 1))
                src = slot[:].rearrange("p (q h d) -> p q h d", q=4, h=NH)
                vbr = vb[:].rearrange("p (q h d) -> p q h d", q=4, h=NH)
                nc.vector.tensor_tensor(
                    out=vsb_r[:, tq * 4:(tq + 1) * 4, :, 64:128], in0=src[:],
                    in1=vbr[:], op=ALU.add)

            # ================= Phase 1: LayerNorm (tb-pipelined) ===========
            x2 = xn  # scratch: tb-slices of x2 are read before xn overwrites
            with tc.tile_pool(name="lnw", bufs=2) as lnwp:
                for tb in range(4):
                    tsl = slice(tb * 512, (tb + 1) * 512)
                    for c in range(DC):
                        sl = slice(c * T + tb * 512, c * T + (tb + 1) * 512)
                        nc.scalar.activation(x2[:, sl], xtb[:, sl], AF.Square)
                    slot = ps(["psC", "psD"][tb % 2], 1024)
                    s_ps, q_ps = slot[:, 0:512], slot[:, 512:1024]
                    for c in range(DC):
                        sl = slice(c * T + tb * 512, c * T + (tb + 1) * 512)
                        nc.tensor.matmul(s_ps, ones128[:], xtb[:, sl],
                                         start=(c == 0), stop=(c == DC - 1))
                    for c in range(DC):
                        sl = slice(c * T + tb * 512, c * T + (tb + 1) * 512)
                        nc.tensor.matmul(q_ps, ones128[:], x2[:, sl],
                                         start=(c == 0), stop=(c == DC - 1))
                    nc.vector.tensor_scalar(out=mean_b[:, tsl], in0=s_ps,
                                            scalar1=1.0 / D, scalar2=None,
                                            op0=ALU.mult)
                    m2 = lnwp.tile([128, 512], fp32, tag="lnm2")
                    nc.vector.tensor_tensor(out=m2[:], in0=mean_b[:, tsl],
                                            in1=mean_b[:, tsl], op=ALU.mult)
                    var = lnwp.tile([128, 512], fp32, tag="lnvar")
                    nc.vector.scalar_tensor_tensor(
                        out=var[:], in0=q_ps, scalar=1.0 / D, in1=m2[:],
                        op0=ALU.mult, op1=ALU.subtract)
                    rcv = lnwp.tile([128, 512], fp32, tag="lnrcv")
                    nc.vector.reciprocal_approx_fast(out=rcv[:], in_=var[:])
                    nc.scalar.activation(rstd_b[:, tsl], rcv[:], AF.Sqrt)
                    for c in range(DC):
                        sl = slice(c * T + tb * 512, c * T + (tb + 1) * 512)
                        eng = nc.vector if c < 6 else nc.gpsimd
                        xc = lnwp.tile([128, 512], bf16, tag=f"lnxc{c % 2}")
                        eng.tensor_tensor(out=xc[:], in0=xtb[:, sl],
                                          in1=mean_b[:, tsl], op=ALU.subtract)
                        eng.tensor_tensor(out=xn[:, sl], in0=xc[:],
                                          in1=rstd_b[:, tsl], op=ALU.mult)
                    # k (m=2) then q (m=0) for this tb, in psA sub-slots
                    a = ps("psA", 2048)
                    qk_unit(2, tb, a[:, (tb % 2) * 1024:(tb % 2) * 1024 + 512])
                    qk_unit(0, tb, a[:, (tb % 2) * 1024 + 512:(tb % 2 + 1) * 1024])
            lnp.__exit__(None, None, None)

            # ============ Phase 2: attention + output projection ============
            with tc.tile_pool(name="attn", bufs=1) as attnp:
                eblk0 = attnp.tile([128, KC * 1024], bf16, tag="eblk0")
                eblk1 = attnp.tile([128, KC * 1024], bf16, tag="eblk1")

                def normalize(blk):
                    qb, pair, av = blk
                    csl = slice(pair * T + qb * QW, pair * T + (qb + 1) * QW)
                    # dens at av row 0: [den_even(0:512) | den_odd(512:1024)]
                    rc = workp.tile([128, 1024], fp32, tag="recf")
                    nc.vector.reciprocal_approx_fast(
                        out=rc[0:1, :], in_=av[0:1, :])
                    rcb = workp.tile([128, 1024], bf16, tag="recb")
                    nc.vector.tensor_copy(out=rcb[0:1, :], in_=rc[0:1, :])
                    rbc = workp.tile([128, 1024], bf16, tag="rbcs")
                    nc.gpsimd.partition_broadcast(rbc[:], rcb[0:1, :])
                    # even head V at av rows 64:128 (free 0:512) -> aot 64:128
                    nc.vector.tensor_tensor(
                        out=aot[64:128, csl], in0=av[64:128, 0:512],
                        in1=rbc[64:128, 0:512], op=ALU.mult)
                    # odd head V at av rows 64:128 (free 512:1024) -> aot 0:64
                    # (aligned cross-base: PSUM in0 base 64, out/in1 base 0)
                    nc.vector.tensor_tensor(
                        out=aot[0:64, csl], in0=av[64:128, 512:1024],
                        in1=rbc[0:64, 512:1024], op=ALU.mult)

                def outproj_grp(qb, mp, tag="psD"):
                    qsl = slice(qb * QW, (qb + 1) * QW)
                    slot = ps(tag, 1024)
                    for half in range(2):
                        m = 2 * mp + half
                        o = slot[:, half * 512:(half + 1) * 512]
                        for c2 in range(2):
                            nc.tensor.matmul(
                                o,
                                wout[:, c2 * D + m * 128:c2 * D + (m + 1) * 128],
                                aot[:, c2 * T + qb * QW:c2 * T + (qb + 1) * QW],
                                start=(c2 == 0), stop=(c2 == 1))
                    ob = workp.tile([128, 1024], bf16, tag="ob")
                    for half in range(2):
                        m = 2 * mp + half
                        nc.vector.tensor_scalar(
                            out=ob[:, half * 512:(half + 1) * 512],
                            in0=slot[:, half * 512:(half + 1) * 512],
                            scalar1=bout[:, m:m + 1], scalar2=None,
                            op0=ALU.add)
                    for half in range(2):
                        m = 2 * mp + half
                        nc.sync.dma_start(
                            out_d[m * 128:(m + 1) * 128, qsl],
                            ob[:, half * 512:(half + 1) * 512])

                fillers = [lambda tq=tq: v_unit(tq, ps("psD", 1024))
                           for tq in range(4)]
                fillers += [lambda m=m, tb=tb: qk_unit(
                    m, tb, ps("psD", 1024)[:, 0:512])
                    for m in (3, 1) for tb in range(4)]
                block_order = [(0, 0), (1, 0), (0, 1), (1, 1),
                               (2, 0), (2, 1), (3, 0), (3, 1)]
                prev = None
                pending_norm = None
                for bi, (qb, pair) in enumerate(block_order):
                    qsl = slice(qb * QW, (qb + 1) * QW)
                    eblk = (eblk0, eblk1)[bi % 2]
                    qm = qk[:, (0 + pair) * T:(1 + pair) * T]
                    km = qk[:, (2 + pair) * T:(3 + pair) * T]
                    # normalize of block bi-2 runs now: its AV (accumulated
                    # during bi-1) is complete, and it must release the psC
                    # slot before this block's AV matmuls start.
                    if pending_norm is not None:
                        normalize(pending_norm)
                        pending_norm = None
                    if prev is not None:
                        pqb, ppair, peblk = prev
                        pav = ps("psC", 1024)

                        def av_mms(c, av=pav, pair=ppair, eblk=peblk):
                            for h in range(2):
                                hh = pair * 2 + h
                                nc.tensor.matmul(
                                    av[:, h * 512:(h + 1) * 512],
                                    vsb[:, (c * NH + hh) * 128:(c * NH + hh + 1) * 128],
                                    eblk[:, c * 1024 + h * 512:c * 1024 + (h + 1) * 512],
                                    start=(c == 0), stop=(c == KC - 1))
                    eps_ = ps("psA", 2048)
                    for g in range(KC // 2):
                        gcad = 2 if bi < 2 else 3
                        if g % gcad == 0 and fillers:
                            fillers.pop(0)()
                        for ci in range(2):
                            c = 2 * g + ci
                            if prev is not None:
                                av_mms(c)
                            ksl = slice(c * 128, (c + 1) * 128)
                            half = eps_[:, (c % 2) * 1024:(c % 2) * 1024 + 1024]
                            nc.tensor.matmul(half[:, 0:512],
                                             km[0:64, ksl], qm[0:64, qsl],
                                             tile_position=(0, 0))
                            nc.tensor.matmul(half[:, 512:1024],
                                             km[64:128, ksl], qm[64:128, qsl],
                                             tile_position=(64, 0))
                            nc.scalar.activation(
                                eblk[:, c * 1024:(c + 1) * 1024], half,
                                AF.Exp)
                    if prev is not None:
                        pending_norm = (pqb, ppair, pav)
                        if ppair == 1:
                            fillers.extend(
                                [lambda q=pqb, mp=mp: outproj_grp(q, mp)
                                 for mp in range(4)])
                    prev = (qb, pair, eblk)
                # tail: AV + normalize of the last block, remaining fillers
                if pending_norm is not None:
                    normalize(pending_norm)
                    pending_norm = None
                pqb, ppair, peblk = prev
                pav = ps("psC", 1024)
                for c in range(KC):
                    for h in range(2):
                        hh = ppair * 2 + h
                        nc.tensor.matmul(
                            pav[:, h * 512:(h + 1) * 512],
                            vsb[:, (c * NH + hh) * 128:(c * NH + hh + 1) * 128],
                            peblk[:, c * 1024 + h * 512:c * 1024 + (h + 1) * 512],
                            start=(c == 0), stop=(c == KC - 1))
                    if c % 3 == 2 and fillers:
                        fillers.pop(0)()
                while fillers:
                    fillers.pop(0)()
                normalize((pqb, ppair, pav))
                for mp in range(4):
                    outproj_grp(pqb, mp, tag=["psD", "psA"][mp % 2])
                if dbg:
                    nc.sync.dma_start(dbg["xn"][:], xn[:])
                    nc.sync.dma_start(dbg["qk"][:], qk[:])
                    nc.sync.dma_start(dbg["vsb"][:], vsb[:])
                    nc.sync.dma_start(dbg["aot"][:], aot[:])

    nc.compile()
    return nc


def _prep_inputs(x, gamma, beta, w_qkv, w_out, b_out):
    import ml_dtypes

    bf16 = ml_dtypes.bfloat16
    wg = (w_qkv * gamma[:, None]).astype(np.float32)  # fold gamma
    bias_full = (beta @ w_qkv).astype(np.float32)  # fold beta
    in_maps = []
    for core in range(NCORES):
        b, g = divmod(core, 4)
        cs = slice(g * 256, (g + 1) * 256)
        qc = wg[:, 0 * D:1 * D][:, cs] * SCALE
        kc = wg[:, 1 * D:2 * D][:, cs]
        vc = wg[:, 2 * D:3 * D][:, cs]
        bq = bias_full[0 * D:1 * D][cs] * SCALE
        bk = bias_full[1 * D:2 * D][cs]
        bv = bias_full[2 * D:3 * D][cs]
        # q/k m-major: [m(4), c(8), 128] from [1024, 512] feature blocks
        wqk = np.concatenate([qc, kc], axis=1)  # [1024, 512]
        wqk_sb = np.zeros((128, WQK), np.float32)
        for m in range(4):
            for c in range(DC):
                wqk_sb[:, (m * DC + c) * 128:(m * DC + c + 1) * 128] = \
                    wqk[c * 128:(c + 1) * 128, m * 128:(m + 1) * 128]
        # v c-major: [c(8), 256]
        wv_sb = vc.reshape(DC, 128, 256).transpose(1, 0, 2).reshape(128, DC * 256)
        wqkv_sb = np.concatenate([wqk_sb, wv_sb], axis=1)  # [128, DC*F]
        xt = np.ascontiguousarray(x[b].T)  # [1024, 2048]
        xt_sb = xt.reshape(DC, 128, T).transpose(1, 0, 2).reshape(128, DC * T)
        wout_core = w_out[g * 256:(g + 1) * 256, :]  # [256, 1024]
        # aot chunk c2: rows 0:64 = odd head (2c2+1), rows 64:128 = even (2c2)
        chunks = []
        for c2 in range(2):
            odd = wout_core[(2 * c2 + 1) * 64:(2 * c2 + 2) * 64, :]
            even = wout_core[(2 * c2) * 64:(2 * c2 + 1) * 64, :]
            chunks.append(np.concatenate([odd, even], axis=0))
        wout_sb = np.concatenate(chunks, axis=1)  # [128, 2D]
        in_maps.append({
            "xt": np.ascontiguousarray(xt_sb).astype(bf16),
            "wqkv": np.ascontiguousarray(wqkv_sb).astype(bf16),
            "wout": np.ascontiguousarray(wout_sb).astype(bf16),
            "bqkc": np.ascontiguousarray(
                np.concatenate([bq, bk]).reshape(4, 128).T).astype(np.float32),
            "vbrow": np.ascontiguousarray(np.tile(bv, 4)[None, :]).astype(bf16),
            "bout": np.ascontiguousarray(b_out.reshape(8, 128).T).astype(np.float32),
        })
    return in_maps


def kernel(x, gamma, beta, w_qkv, w_out, b_out, _want_trace=False):
    from concourse.bass_utils import run_bass_kernel_spmd

    x = np.asarray(x, dtype=np.float32)
    gamma = np.asarray(gamma, dtype=np.float32)
    beta = np.asarray(beta, dtype=np.float32)
    w_qkv = np.asarray(w_qkv, dtype=np.float32)
    w_out = np.asarray(w_out, dtype=np.float32)
    b_out = np.asarray(b_out, dtype=np.float32)

    if "nc" not in _cache:
        _cache["nc"] = _build()
    nc = _cache["nc"]
    in_maps = _prep_inputs(x, gamma, beta, w_qkv, w_out, b_out)
    res = run_bass_kernel_spmd(nc, in_maps, core_ids=list(range(NCORES)),
                               trace=_want_trace)
    _cache["last_result"] = res
    out = np.empty((B, S, D), dtype=np.float32)
    for b in range(B):
        acc = np.zeros((D, T), dtype=np.float32)
        for g in range(4):
            acc += res.results[b * 4 + g]["out"].astype(np.float32)
        out[b] = acc.T
    return out


# revision 10
# speedup vs baseline: 1.1830x; 1.1830x over previous
"""Fused LayerNorm + multi-head attention + output projection on 8 TRN2 cores.

Sharding: core c handles batch b = c//4 and head group g = c%4 (4 of 16 heads).
Each core computes LN(x[b]) (replicated within the batch's 4 cores), the qkv
projection for its heads, attention, and a partial output projection (w_out
rows for its heads). The host sums the 4 partials per batch.

On-chip layout is fully transposed ([feature, token]); the host pre-transposes
x, folds gamma / softmax scale / beta into the weights, and packs everything in
SBUF-ready layouts, so the kernel needs zero on-chip transposes:

  xn^T   [D, T]   = LayerNorm(x)^T        (stats via ones-matmul broadcast,
                                           tb-pipelined with the qkv matmuls)
  q^T/k^T [dh, T] = W_q/k^T-slices @ xn^T (feature-major)
  v      [T, dh]  = xn^T-tiles.T @ W_v    (token-major, swapped operands;
                                           v-bias folded into the V columns)
  E^T    [k, q]   = exp(K Q^T)            (no max subtraction: scores ~N(0,1))
  av^T   [dh, q]  = V-block @ E^T         (den rows ride along: even head ->
                                           psum row 127, odd head -> row 0)
  out^T  [D, T]   = w_out-slice^T @ (av^T/den)   (2 heads packed per 128 rows)
"""

import numpy as np

HEADS = 16
DIM_HEAD = 64
SCALE = DIM_HEAD**-0.5
EPS = 1e-5
B, S, D = 2, 2048, 1024
T = S
NCORES = 8
NH = 4  # heads per core
F = 3 * NH * DIM_HEAD  # 768 features per core: [q(256) | k(256) | v(256)]
DC = D // 128  # 8 contraction chunks
KC = T // 128  # 16 key chunks
QB = 4  # q blocks
QW = T // QB  # 512 q block width
WQK = 4 * DC * 128  # 4096: m-major q/k region of the wqkv tile

_cache = {}


def _build():
    import concourse.bacc as bacc
    import concourse.mybir as mybir
    import concourse.tile as tile

    fp32 = mybir.dt.float32
    bf16 = mybir.dt.bfloat16
    AF = mybir.ActivationFunctionType
    ALU = mybir.AluOpType

    nc = bacc.Bacc("TRN2", target_bir_lowering=False, debug=False,
                   num_devices=NCORES)
    # wqkv layout: [q/k m-major: (m c j) 4*8*128 | v c-major: (c j) 8*256]
    xt_d = nc.declare_dram_parameter("xt", [128, DC * T], bf16, isOutput=False)
    wqkv_d = nc.declare_dram_parameter("wqkv", [128, DC * F], bf16, isOutput=False)
    wout_d = nc.declare_dram_parameter("wout", [128, 2 * D], bf16, isOutput=False)
    bqkc_d = nc.declare_dram_parameter("bqkc", [128, 4], fp32, isOutput=False)
    vbrow_d = nc.declare_dram_parameter("vbrow", [1, 1024], bf16, isOutput=False)
    bout_d = nc.declare_dram_parameter("bout", [128, 8], fp32, isOutput=False)
    out_d = nc.declare_dram_parameter("out", [D, T], bf16, isOutput=True)
    dbg = {}
    if _cache.get("debug"):
        dbg["xn"] = nc.declare_dram_parameter("dbg_xn", [128, DC * T], bf16, isOutput=True)
        dbg["qk"] = nc.declare_dram_parameter("dbg_qk", [128, 4 * T], bf16, isOutput=True)
        dbg["vsb"] = nc.declare_dram_parameter("dbg_vsb", [128, KC * NH * 128], bf16, isOutput=True)
        dbg["aot"] = nc.declare_dram_parameter("dbg_aot", [128, 2 * T], bf16, isOutput=True)

    with tile.TileContext(nc) as tc:
        with (
            tc.tile_pool(name="const", bufs=1) as constp,
            tc.tile_pool(name="big", bufs=1) as bigp,
            tc.tile_pool(name="work", bufs=2) as workp,
            tc.tile_pool(name="psum", bufs=1, space="PSUM") as psump,
        ):
            # ---- persistent SBUF ----
            ones128 = constp.tile([128, 128], bf16, tag="ones128")
            nc.gpsimd.memset(ones128[:], 1.0)
            wqkv = constp.tile([128, DC * F], bf16, tag="wqkv")
            wout = constp.tile([128, 2 * D], bf16, tag="wout")
            bqkc = constp.tile([128, 4], fp32, tag="bqkc")
            bout = constp.tile([128, 8], fp32, tag="bout")
            vbrow = constp.tile([1, 1024], bf16, tag="vbrow")
            vb = constp.tile([128, 1024], bf16, tag="vb")

            xn = bigp.tile([128, DC * T], bf16, tag="xn")  # normalized x^T
            mean_b = bigp.tile([128, T], bf16, tag="mean_b")
            rstd_b = bigp.tile([128, T], bf16, tag="rstd_b")
            # q^T / k^T feature-major: m=0,1 -> q heads (0,1),(2,3); m=2,3 -> k
            qk = bigp.tile([128, 4 * T], bf16, tag="qk")
            # v blocks, 128 wide per (k-chunk, head), all heads alike:
            #   [one@0 | zeros(63) | V(64)@64:128] -> den at av row 0
            vsb = bigp.tile([128, KC * NH * 128], bf16, tag="vsb")
            nc.gpsimd.memset(vsb[:], 0.0)
            vsb_r = vsb[:].rearrange("p (c h o) -> p c h o", h=NH, o=128)
            nc.gpsimd.memset(vsb_r[:, :, :, 0:1], 1.0)
            # attention output^T, packed: chunk hh//2; odd head -> rows 0:64,
            # even head -> rows 64:128 (wout_sb rows swapped to match)
            aot = bigp.tile([128, 2 * T], bf16, tag="aot")

            # psum: A1/A2 = scores ping-pong / phase-1 qkv (2+2 banks),
            #       C = AV / phase-1 stats (2), D = fillers / stats (2)
            ps_n = [0]

            def ps(tag, width):
                ps_n[0] += 1
                return psump.tile([128, width], fp32, tag=tag,
                                  name=f"ps_{tag}_{ps_n[0]}")

            # ---- input DMAs, interleaved for just-in-time arrival ----
            lnp = tc.tile_pool(name="ln", bufs=1)
            lnp_pool = lnp.__enter__()
            xtb = lnp_pool.tile([128, DC * T], bf16, tag="xtb")

            def dma_xt(tb):
                for c in range(DC):
                    sl = slice(c * T + tb * 512, c * T + (tb + 1) * 512)
                    nc.sync.dma_start(xtb[:, sl], xt_d[:, sl])

            def dma_w(lo, hi):
                nc.sync.dma_start(wqkv[:, lo:hi], wqkv_d[:, lo:hi])

            nc.sync.dma_start(bqkc[:], bqkc_d[:])
            nc.sync.dma_start(vbrow[:], vbrow_d[:])
            dma_xt(0)
            dma_w(2 * 1024, 3 * 1024)  # m=2 (k heads 0,1)
            dma_w(0 * 1024, 1 * 1024)  # m=0 (q heads 0,1)
            dma_xt(1)
            dma_w(3 * 1024, 4 * 1024)  # m=3
            dma_w(1 * 1024, 2 * 1024)  # m=1
            dma_xt(2)
            nc.sync.dma_start(wqkv[:, WQK:], wqkv_d[:, WQK:])  # v
            dma_xt(3)
            nc.sync.dma_start(wout[:], wout_d[:])
            nc.sync.dma_start(bout[:], bout_d[:])
            nc.gpsimd.partition_broadcast(vb[:], vbrow[0:1, :])

            # ---- qkv building blocks ----
            def qk_unit(m, tb, slot):
                # q/k projection for one (m-slice, token-block): [128, 512]
                tsl = slice(tb * 512, (tb + 1) * 512)
                for c in range(DC):
                    nc.tensor.matmul(
                        slot,
                        wqkv[:, (m * DC + c) * 128:(m * DC + c + 1) * 128],
                        xn[:, c * T + tb * 512:c * T + (tb + 1) * 512],
                        start=(c == 0), stop=(c == DC - 1))
                nc.vector.tensor_scalar(
                    out=qk[:, m * T + tb * 512:m * T + (tb + 1) * 512],
                    in0=slot, scalar1=bqkc[:, m:m + 1], scalar2=None,
                    op0=ALU.add)

            def v_unit(tq, slot):
                # v for 4 token-tiles (512 tokens), token-major [tok, (h d)]
                for half in range(4):
                    tt = tq * 4 + half
                    o = slot[:, half * 256:(half + 1) * 256]
                    for c in range(DC):
                        nc.tensor.matmul(
                            o,
                            xn[:, c * T + tt * 128:c * T + (tt + 1) * 128],
                            wqkv[:, WQK + c * 256:WQK + (c + 1) * 256],
                            start=(c == 0), stop=(c == DC - 1))
                src = slot[:].rearrange("p (q h d) -> p q h d", q=4, h=NH)
                vbr = vb[:].rearrange("p (q h d) -> p q h d", q=4, h=NH)
                nc.vector.tensor_tensor(
                    out=vsb_r[:, tq * 4:(tq + 1) * 4, :, 64:128], in0=src[:],
                    in1=vbr[:], op=ALU.add)

            # ================= Phase 1: LayerNorm (tb-pipelined) ===========
            x2 = xn  # scratch: tb-slices of x2 are read before xn overwrites
            with tc.tile_pool(name="lnw", bufs=2) as lnwp:
                for tb in range(4):
                    tsl = slice(tb * 512, (tb + 1) * 512)
                    for c in range(DC):
                        sl = slice(c * T + tb * 512, c * T + (tb + 1) * 512)
                        if c < 6:
                            nc.scalar.activation(x2[:, sl], xtb[:, sl],
                                                 AF.Square)
                        else:
                            nc.gpsimd.tensor_tensor(out=x2[:, sl],
                                                    in0=xtb[:, sl],
                                                    in1=xtb[:, sl],
                                                    op=ALU.mult)
                    slot = ps(["psC", "psD"][tb % 2], 1024)
                    s_ps, q_ps = slot[:, 0:512], slot[:, 512:1024]
                    for c in range(DC):
                        sl = slice(c * T + tb * 512, c * T + (tb + 1) * 512)
                        nc.tensor.matmul(s_ps, ones128[:], xtb[:, sl],
                                         start=(c == 0), stop=(c == DC - 1))
                    for c in range(DC):
                        sl = slice(c * T + tb * 512, c * T + (tb + 1) * 512)
                        nc.tensor.matmul(q_ps, ones128[:], x2[:, sl],
                                         start=(c == 0), stop=(c == DC - 1))
                    nc.vector.tensor_scalar(out=mean_b[:, tsl], in0=s_ps,
                                            scalar1=1.0 / D, scalar2=None,
                                            op0=ALU.mult)
                    m2 = lnwp.tile([128, 512], fp32, tag="lnm2")
                    nc.vector.tensor_tensor(out=m2[:], in0=mean_b[:, tsl],
                                            in1=mean_b[:, tsl], op=ALU.mult)
                    var = lnwp.tile([128, 512], fp32, tag="lnvar")
                    nc.vector.scalar_tensor_tensor(
                        out=var[:], in0=q_ps, scalar=1.0 / D, in1=m2[:],
                        op0=ALU.mult, op1=ALU.subtract)
                    rcv = lnwp.tile([128, 512], fp32, tag="lnrcv")
                    nc.vector.reciprocal_approx_fast(out=rcv[:], in_=var[:])
                    nc.scalar.activation(rstd_b[:, tsl], rcv[:], AF.Sqrt)
                    for c in range(DC):
                        sl = slice(c * T + tb * 512, c * T + (tb + 1) * 512)
                        eng = nc.vector if c < 6 else nc.gpsimd
                        xc = lnwp.tile([128, 512], bf16, tag=f"lnxc{c % 2}")
                        eng.tensor_tensor(out=xc[:], in0=xtb[:, sl],
                                          in1=mean_b[:, tsl], op=ALU.subtract)
                        eng.tensor_tensor(out=xn[:, sl], in0=xc[:],
                                          in1=rstd_b[:, tsl], op=ALU.mult)
                    # k (m=2) then q (m=0) for this tb, in psA sub-slots
                    qk_unit(2, tb, ps("psA1", 1024)[:, 0:512])
                    qk_unit(0, tb, ps("psA2", 1024)[:, 0:512])
            lnp.__exit__(None, None, None)

            # ============ Phase 2: attention + output projection ============
            with tc.tile_pool(name="attn", bufs=1) as attnp:
                eblk0 = attnp.tile([128, KC * 1024], bf16, tag="eblk0")
                eblk1 = attnp.tile([128, KC * 1024], bf16, tag="eblk1")

                def normalize(blk):
                    qb, pair, av = blk
                    csl = slice(pair * T + qb * QW, pair * T + (qb + 1) * QW)
                    # dens at av row 0: [den_even(0:512) | den_odd(512:1024)]
                    rc = workp.tile([128, 1024], fp32, tag="recf")
                    nc.vector.reciprocal_approx_fast(
                        out=rc[0:1, :], in_=av[0:1, :])
                    rcb = workp.tile([128, 1024], bf16, tag="recb")
                    nc.vector.tensor_copy(out=rcb[0:1, :], in_=rc[0:1, :])
                    rbc = workp.tile([128, 1024], bf16, tag="rbcs")
                    nc.gpsimd.partition_broadcast(rbc[:], rcb[0:1, :])
                    # even head V at av rows 64:128 (free 0:512) -> aot 64:128
                    nc.vector.tensor_tensor(
                        out=aot[64:128, csl], in0=av[64:128, 0:512],
                        in1=rbc[64:128, 0:512], op=ALU.mult)
                    # odd head V at av rows 64:128 (free 512:1024) -> aot 0:64
                    # (aligned cross-base: PSUM in0 base 64, out/in1 base 0)
                    nc.vector.tensor_tensor(
                        out=aot[0:64, csl], in0=av[64:128, 512:1024],
                        in1=rbc[0:64, 512:1024], op=ALU.mult)

                def outproj_grp(qb, mp, tag="psD"):
                    qsl = slice(qb * QW, (qb + 1) * QW)
                    slot = ps(tag, 1024)
                    for half in range(2):
                        m = 2 * mp + half
                        o = slot[:, half * 512:(half + 1) * 512]
                        for c2 in range(2):
                            nc.tensor.matmul(
                                o,
                                wout[:, c2 * D + m * 128:c2 * D + (m + 1) * 128],
                                aot[:, c2 * T + qb * QW:c2 * T + (qb + 1) * QW],
                                start=(c2 == 0), stop=(c2 == 1))
                    ob = workp.tile([128, 1024], bf16, tag="ob")
                    for half in range(2):
                        m = 2 * mp + half
                        nc.vector.tensor_scalar(
                            out=ob[:, half * 512:(half + 1) * 512],
                            in0=slot[:, half * 512:(half + 1) * 512],
                            scalar1=bout[:, m:m + 1], scalar2=None,
                            op0=ALU.add)
                    for half in range(2):
                        m = 2 * mp + half
                        nc.sync.dma_start(
                            out_d[m * 128:(m + 1) * 128, qsl],
                            ob[:, half * 512:(half + 1) * 512])

                fillers = [lambda tq=tq: v_unit(tq, ps("psD", 1024))
                           for tq in range(4)]
                fillers += [lambda m=m, tb=tb: qk_unit(
                    m, tb, ps("psD", 1024)[:, 0:512])
                    for m in (3, 1) for tb in range(4)]
                block_order = [(0, 0), (1, 0), (0, 1), (1, 1),
                               (2, 0), (2, 1), (3, 0), (3, 1)]
                prev = None
                pending_norm = None
                for bi, (qb, pair) in enumerate(block_order):
                    qsl = slice(qb * QW, (qb + 1) * QW)
                    eblk = (eblk0, eblk1)[bi % 2]
                    qm = qk[:, (0 + pair) * T:(1 + pair) * T]
                    km = qk[:, (2 + pair) * T:(3 + pair) * T]
                    # normalize of block bi-2 runs now: its AV (accumulated
                    # during bi-1) is complete, and it must release the psC
                    # slot before this block's AV matmuls start.
                    if pending_norm is not None:
                        normalize(pending_norm)
                        pending_norm = None
                    if prev is not None:
                        pqb, ppair, peblk = prev
                        pav = ps("psC", 1024)

                        def av_mms(c, av=pav, pair=ppair, eblk=peblk):
                            for h in range(2):
                                hh = pair * 2 + h
                                nc.tensor.matmul(
                                    av[:, h * 512:(h + 1) * 512],
                                    vsb[:, (c * NH + hh) * 128:(c * NH + hh + 1) * 128],
                                    eblk[:, c * 1024 + h * 512:c * 1024 + (h + 1) * 512],
                                    start=(c == 0), stop=(c == KC - 1))
                    for g in range(KC // 2):
                        gcad = 2 if bi < 2 else 3
                        if g % gcad == 0 and fillers:
                            fillers.pop(0)()
                        for ci in range(2):
                            c = 2 * g + ci
                            if prev is not None:
                                av_mms(c)
                            ksl = slice(c * 128, (c + 1) * 128)
                            half = ps(["psA1", "psA2"][c % 2], 1024)
                            nc.tensor.matmul(half[:, 0:512],
                                             km[0:64, ksl], qm[0:64, qsl],
                                             tile_position=(0, 0))
                            nc.tensor.matmul(half[:, 512:1024],
                                             km[64:128, ksl], qm[64:128, qsl],
                                             tile_position=(64, 0))
                            nc.scalar.activation(
                                eblk[:, c * 1024:(c + 1) * 1024], half,
                                AF.Exp)
                    if prev is not None:
                        pending_norm = (pqb, ppair, pav)
                        if ppair == 1:
                            fillers.extend(
                                [lambda q=pqb, mp=mp: outproj_grp(q, mp)
                                 for mp in range(4)])
                    prev = (qb, pair, eblk)
                # tail: AV + normalize of the last block, remaining fillers
                if pending_norm is not None:
                    normalize(pending_norm)
                    pending_norm = None
                pqb, ppair, peblk = prev
                pav = ps("psC", 1024)
                for c in range(KC):
                    for h in range(2):
                        hh = ppair * 2 + h
                        nc.tensor.matmul(
                            pav[:, h * 512:(h + 1) * 512],
                            vsb[:, (c * NH + hh) * 128:(c * NH + hh + 1) * 128],
                            peblk[:, c * 1024 + h * 512:c * 1024 + (h + 1) * 512],
                            start=(c == 0), stop=(c == KC - 1))
                    if c % 3 == 2 and fillers:
                        fillers.pop(0)()
                while fillers:
                    fillers.pop(0)()
                normalize((pqb, ppair, pav))
                for mp in range(4):
                    outproj_grp(pqb, mp, tag=["psD", "psA1"][mp % 2])
                if dbg:
                    nc.sync.dma_start(dbg["xn"][:], xn[:])
                    nc.sync.dma_start(dbg["qk"][:], qk[:])
                    nc.sync.dma_start(dbg["vsb"][:], vsb[:])
                    nc.sync.dma_start(dbg["aot"][:], aot[:])

    nc.compile()
    return nc


def _prep_inputs(x, gamma, beta, w_qkv, w_out, b_out):
    import ml_dtypes

    bf16 = ml_dtypes.bfloat16
    wg = (w_qkv * gamma[:, None]).astype(np.float32)  # fold gamma
    bias_full = (beta @ w_qkv).astype(np.float32)  # fold beta
    in_maps = []
    for core in range(NCORES):
        b, g = divmod(core, 4)
        cs = slice(g * 256, (g + 1) * 256)
        qc = wg[:, 0 * D:1 * D][:, cs] * SCALE
        kc = wg[:, 1 * D:2 * D][:, cs]
        vc = wg[:, 2 * D:3 * D][:, cs]
        bq = bias_full[0 * D:1 * D][cs] * SCALE
        bk = bias_full[1 * D:2 * D][cs]
        bv = bias_full[2 * D:3 * D][cs]
        # q/k m-major: [m(4), c(8), 128] from [1024, 512] feature blocks
        wqk = np.concatenate([qc, kc], axis=1)  # [1024, 512]
        wqk_sb = np.zeros((128, WQK), np.float32)
        for m in range(4):
            for c in range(DC):
                wqk_sb[:, (m * DC + c) * 128:(m * DC + c + 1) * 128] = \
                    wqk[c * 128:(c + 1) * 128, m * 128:(m + 1) * 128]
        # v c-major: [c(8), 256]
        wv_sb = vc.reshape(DC, 128, 256).transpose(1, 0, 2).reshape(128, DC * 256)
        wqkv_sb = np.concatenate([wqk_sb, wv_sb], axis=1)  # [128, DC*F]
        xt = np.ascontiguousarray(x[b].T)  # [1024, 2048]
        xt_sb = xt.reshape(DC, 128, T).transpose(1, 0, 2).reshape(128, DC * T)
        wout_core = w_out[g * 256:(g + 1) * 256, :]  # [256, 1024]
        # aot chunk c2: rows 0:64 = odd head (2c2+1), rows 64:128 = even (2c2)
        chunks = []
        for c2 in range(2):
            odd = wout_core[(2 * c2 + 1) * 64:(2 * c2 + 2) * 64, :]
            even = wout_core[(2 * c2) * 64:(2 * c2 + 1) * 64, :]
            chunks.append(np.concatenate([odd, even], axis=0))
        wout_sb = np.concatenate(chunks, axis=1)  # [128, 2D]
        in_maps.append({
            "xt": np.ascontiguousarray(xt_sb).astype(bf16),
            "wqkv": np.ascontiguousarray(wqkv_sb).astype(bf16),
            "wout": np.ascontiguousarray(wout_sb).astype(bf16),
            "bqkc": np.ascontiguousarray(
                np.concatenate([bq, bk]).reshape(4, 128).T).astype(np.float32),
            "vbrow": np.ascontiguousarray(np.tile(bv, 4)[None, :]).astype(bf16),
            "bout": np.ascontiguousarray(b_out.reshape(8, 128).T).astype(np.float32),
        })
    return in_maps


def kernel(x, gamma, beta, w_qkv, w_out, b_out, _want_trace=False):
    from concourse.bass_utils import run_bass_kernel_spmd

    x = np.asarray(x, dtype=np.float32)
    gamma = np.asarray(gamma, dtype=np.float32)
    beta = np.asarray(beta, dtype=np.float32)
    w_qkv = np.asarray(w_qkv, dtype=np.float32)
    w_out = np.asarray(w_out, dtype=np.float32)
    b_out = np.asarray(b_out, dtype=np.float32)

    if "nc" not in _cache:
        _cache["nc"] = _build()
    nc = _cache["nc"]
    in_maps = _prep_inputs(x, gamma, beta, w_qkv, w_out, b_out)
    res = run_bass_kernel_spmd(nc, in_maps, core_ids=list(range(NCORES)),
                               trace=_want_trace)
    _cache["last_result"] = res
    out = np.empty((B, S, D), dtype=np.float32)
    for b in range(B):
        acc = np.zeros((D, T), dtype=np.float32)
        for g in range(4):
            acc += res.results[b * 4 + g]["out"].astype(np.float32)
        out[b] = acc.T
    return out
